# revision 27
# baseline (speedup 1.0000x reference)
"""Causal local-window (W=128) attention block + FFN, distributed over 8 TRN2
NeuronCores with ZERO collectives.

Sharding: (B=2, L=2048) tokens are split into 8 contiguous segments of 512
tokens (4 per batch element). Each core receives its 512 owned tokens plus a
128-token left halo (zero-padded for the first segment of each batch) and
recomputes the halo's K/V locally — the sliding window (j in [i-128, i]) never
crosses more than 128 tokens back, so no cross-core communication is needed.

Per-core compute layout (v3 — HAM/overlap-optimized):
  - residual stream + LayerNorm stats in token-major [128 tok, 1024] f32
  - matmul activations in feature-major bf16 (PE transposes after each LN)
  - QKV/out-proj/FFN matmuls: bf16 stationary weights, f32 PSUM accumulation;
    bv/bo/b2 biases are folded into the matmuls as a K=1 leading accumulation
    step (ones-row x bias-row), freeing the DVE of broadcast adds.
  - attention: per (head-pair, query-block) two [128,256] f32 PSUM score
    tiles (two single-matmul groups into column ranges of ONE bank hang the
    PE on HW), exp straight out of PSUM (no additive mask), multiplicative
    0/1 mask fused with the row-sum on DVE (scalar_tensor_tensor accum), and
    the softmax 1/rowsum normalization folded into the PE transpose by
    multiplying against diag(rinv) instead of the identity.
  - LN2 + FFN W1 run inside the attention loop: a 16-wide jdff slab of W1
    fires after qb 1, 2 and 3 (x2) as ~14us dense PE bursts — this fills PE
    idle AND re-releases the HAM clock throttle, which otherwise pins the
    whole vector-bound attention phase at K=4/8 (1.2 GHz). W1 streams
    through three 2MB chunk buffers (first pass prefetched, cols 0:2048
    re-streamed for the second token pair: +8MB DMA, well within slack).
  - DMA: every dma_start issues from the gpsimd queue (the SWDGE descriptor
    ring is one shared FIFO per direction; issuing from two engines corrupts
    it and hangs the device). Issue order = arrival order, sized so each
    consumer never waits long.
  - LN scale/bias and the 1/sqrt(dh) score scale are folded into the weight
    matrices on the host, so on-chip LN is pure standardization.
"""

import os
import numpy as np
import ml_dtypes

import concourse.bass as bass
import concourse.mybir as mybir
import concourse.tile as tile
from concourse.masks import make_identity
from bass_rust import ScopedClock

# ---------------------------------------------------------------------------
# Workarounds for the walrus build in this container, which accepts at most
# ONE sync-wait and ONE sync-update per instruction. Tile attaches one wait
# per out-of-date producer clock and one update per consumer engine, so any
# nontrivial Tile kernel violates this. Fix by splitting the extras onto
# standalone InstEventSemaphore instructions on the same engine: waits go
# immediately BEFORE the instruction, updates immediately AFTER (each engine
# executes its stream in order, so semantics are preserved).
_split_counter = [0]


def _split_multi_sync(nc):
    for f in nc.m.functions:
        for bb in f.blocks:
            il = list(bb.instructions)
            new = []
            changed = False
            for inst in il:
                si = inst.sync_info
                waits = list(si.on_wait) if si and si.on_wait else []
                upds = list(si.on_update) if si and si.on_update else []
                if len(waits) > 1:
                    changed = True
                    for w in waits[:-1]:
                        _split_counter[0] += 1
                        new.append(mybir.InstEventSemaphore(
                            name=f"I-wsplit-{_split_counter[0]}",
                            engine=inst.engine, ins=[], outs=[],
                            sync_info=mybir.SyncInfo(on_wait=[w], on_update=[]),
                        ))
                    si.on_wait = [waits[-1]]
                new.append(inst)
                if len(upds) > 1:
                    changed = True
                    si.on_update = [upds[0]]
                    for u in upds[1:]:
                        _split_counter[0] += 1
                        new.append(mybir.InstEventSemaphore(
                            name=f"I-usplit-{_split_counter[0]}",
                            engine=inst.engine, ins=[], outs=[],
                            sync_info=mybir.SyncInfo(on_wait=[], on_update=[u]),
                        ))
            if changed:
                bb.instructions = new


def _patched_drain_and_barrier(self, tick_clock, wait_clock):
    # Tile's kernel-tail drain carries one wait per logical processor; split
    # them into standalone single-wait SP instructions instead.
    nc = self.nc
    drain_inst = nc.sync.drain()
    wait_clock.add_sem_waits(drain_inst.ins, ScopedClock({None: tick_clock.global_clock}))
    si = drain_inst.ins.sync_info
    waits = list(si.on_wait or [])
    if len(waits) > 1:
        si.on_wait = []
        handles = {}
        for s in self.sems.allocated().values():
            nm = getattr(s, 'ant_name', None) or getattr(s, 'name', None)
            handles[nm] = s
        for w in waits:
            assert w.wait_mode == 'sem-ge-imm', w
            nc.sync.wait_ge(handles[w.ant_name], w.wait_value)
    nc.all_engine_barrier()
    assert self.sems is not None
    popped = nc._tile_sem_poison_stack.pop()
    assert popped is self._sem_poison
    nc.clear_and_free_semaphores(list(self.sems.allocated().values()))
    nc.all_engine_barrier()


tile.TileContext._drain_and_barrier = _patched_drain_and_barrier

F32 = mybir.dt.float32
BF16 = mybir.dt.bfloat16
AF = mybir.ActivationFunctionType
ALU = mybir.AluOpType
AX = mybir.AxisListType

B, L, D = 2, 2048, 1024
NH, DH = 16, 64
DFF = 4096
WIN = 128
SEG = 512          # owned tokens per core
HALO = 128
T = SEG + HALO     # 640 local tokens
NT = T // 128      # 5 local token tiles
NSEG = 8           # cores
LN_EPS = 1e-5

_CACHED = {}


def _build():
    nc = bass.Bass()
    # x ships as bf16: halves the critical-path head DMA (~6us); the
    # LN1 stats/apply and the residual adds tolerate 0.4% input rounding
    # easily within the 2e-2 gate (measured ~2.5e-3 end to end).
    x_ext = nc.declare_dram_parameter("x", [T, D], BF16, isOutput=False)
    wq_ext = nc.declare_dram_parameter("wq", [D, D], BF16, isOutput=False)
    wk_ext = nc.declare_dram_parameter("wk", [D, D], BF16, isOutput=False)
    wv_ext = nc.declare_dram_parameter("wv", [D, D], BF16, isOutput=False)
    wo_ext = nc.declare_dram_parameter("wo", [D, D], BF16, isOutput=False)
    w1_ext = nc.declare_dram_parameter("w1", [D, DFF], BF16, isOutput=False)
    w2_ext = nc.declare_dram_parameter("w2", [DFF, D], BF16, isOutput=False)
    bq_ext = nc.declare_dram_parameter("bq", [D], F32, isOutput=False)
    bk_ext = nc.declare_dram_parameter("bk", [D], F32, isOutput=False)
    bv_ext = nc.declare_dram_parameter("bv", [D], BF16, isOutput=False)
    bo_ext = nc.declare_dram_parameter("bo", [D], BF16, isOutput=False)
    b1_ext = nc.declare_dram_parameter("b1", [DFF], F32, isOutput=False)
    b2_ext = nc.declare_dram_parameter("b2", [D], BF16, isOutput=False)
    mask0_ext = nc.declare_dram_parameter("mask0", [128, 512], BF16, isOutput=False)
    maskr_ext = nc.declare_dram_parameter("maskr", [128, 512], BF16, isOutput=False)
    out_ext = nc.declare_dram_parameter("out", [SEG, D], F32, isOutput=True)

    with tile.TileContext(nc) as tc:
        _body(nc, tc, locals())
    if not int(os.environ.get("KERNEL_NO_SPLIT", "0")):
        # walrus-only workaround; CoreSim's race detector rejects the
        # split program, so simcheck.py builds with KERNEL_NO_SPLIT=1.
        _split_multi_sync(nc)
    return nc


def _ln_stats(nc, ln, x_ap, mv_ap):
    """bn stats for one [128, D] f32 tile -> mv_ap [128, 2] (mean, var)."""
    stats = ln.tile([128, 2, 6], F32, tag="ln_stats")
    xr = x_ap.rearrange("p (s f) -> p s f", f=512)
    for s in range(2):
        nc.vector.bn_stats(out=stats[:, s, :], in_=xr[:, s, :])
    nc.vector.bn_aggr(out=mv_ap, in_=stats[:, :, :])


def _ln_apply(nc, ln, x_ap, mv_ap, h_out_ap, eps_tile):
    """h_out = (x - mean) * rsqrt(var + eps), [128, D] f32 -> bf16.

    rstd = exp(-0.5 * ln(var + eps)): Ln and Exp live in the SAME activation
    table set (natural_log_exp_and_others) as the attention-phase Exp, so the
    ACT engine never reloads its function table mid-phase (Sqrt lives in a
    different set and forced a 1.3us ACT_TABLE_LOAD round-trip per LN2 block).
    """
    lnv = ln.tile([128, 1], F32, tag="ln_lnv")
    nc.scalar.activation(out=lnv, in_=mv_ap[:, 1:2], func=AF.Ln, bias=eps_tile, scale=1.0)
    rstd = ln.tile([128, 1], F32, tag="ln_rstd")
    nc.scalar.activation(out=rstd, in_=lnv, func=AF.Exp, bias=0.0, scale=-0.5)
    nmr = ln.tile([128, 1], F32, tag="ln_nmr")
    nc.vector.scalar_tensor_tensor(out=nmr, in0=mv_ap[:, 0:1], scalar=-1.0,
                                   in1=rstd, op0=ALU.mult, op1=ALU.mult)
    nc.scalar.activation(out=h_out_ap, in_=x_ap, func=AF.Identity, bias=nmr, scale=rstd)


def _body(nc, tc, ext):
    st = tc.tile_pool  # shorthand

    with (
        st(name="const", bufs=1) as const,
        st(name="resid", bufs=1) as resid,
        st(name="ln", bufs=3) as ln,
        st(name="scr", bufs=2) as scr,
    ):
        def ptile(pool, shape, tg):
            return pool.tile(shape, F32, tag=tg, name="pst_" + tg)

        def ptile_bf(pool, shape, tg):
            return pool.tile(shape, BF16, tag=tg, name="pstb_" + tg)

        # ---- long-lived tiles ----
        # bf16 residual stream: frees 8KB/partition of SBUF; adds ~0.4%
        # rounding on x2, far inside the 2e-2 gate.
        x2_sb = resid.tile([128, 4, D], BF16)
        mv2 = resid.tile([128, 4, 2], F32)

        ident = const.tile([128, 128], BF16)
        eps_tile = const.tile([128, 1], F32)
        bq_sb = const.tile([128, 8], F32)
        bk_sb = const.tile([128, 8], F32)
        b1_sb = const.tile([128, 32], F32)
        ones1 = const.tile([1, 128], BF16)
        bvrow = const.tile([1, D], BF16)

        with st(name="attnw", bufs=1) as attnw, st(name="fgt", bufs=1) as fgt, \
             st(name="w1c", bufs=3) as w1c:
            x_sb = attnw.tile([128, 4, D], BF16)     # owned tokens only
            qT = attnw.tile([128, 8, SEG], BF16)
            kT = attnw.tile([128, 8, T], BF16)
            v_sb = attnw.tile([128, NT, D], BF16)
            h2T = fgt.tile([128, 8, SEG], BF16)
            gT = fgt.tile([128, 32, SEG], BF16)
            w1r = ext["w1_ext"].rearrange("(k p) n -> p k n", p=128)
            w2r = ext["w2_ext"].rearrange("(c p) n -> p c n", p=128)
            wcnt = [0]
            wtiles = []

            def wchunk_dma(src_ap):
                t_ = w1c.tile([128, 8, 1024], BF16, tag="w1c", name=f"wch{wcnt[0]}")
                wcnt[0] += 1
                nc.gpsimd.dma_start(out=t_, in_=src_ap)
                wtiles.append(t_)


            with (
                st(name="pmm", bufs=2, space="PSUM") as pmm,
                st(name="pscore", bufs=4, space="PSUM") as pscore,
                st(name="ptr", bufs=1, space="PSUM") as ptr,
                st(name="pctx", bufs=1, space="PSUM") as pctx,
                st(name="soft", bufs=4) as soft,
                st(name="atile", bufs=1) as atile,
            ):
                wua = pctx.tile([128, 128], BF16, tag="pctx", name="wua")
                mask0 = atile.tile([128, 512], BF16)
                maskr = atile.tile([128, 512], BF16)
                ctxT = atile.tile([128, 8, SEG], BF16)

                def emit_scores(qb, j2):
                    pss = []
                    for hi, r in enumerate((0, 64)):
                        # separate PSUM tiles: two single-matmul groups
                        # into column ranges of ONE bank hang the PE on
                        # HW (works in sim).
                        ps = ptile(pscore, [128, 256], "psc")
                        nc.tensor.matmul(ps,
                                         qT[r:r + 64, j2, qb * 128:(qb + 1) * 128],
                                         kT[r:r + 64, j2, qb * 128:qb * 128 + 256],
                                         start=True, stop=True)
                        pss.append(ps)
                    return pss

                def j2_body(qb, j2, pss, mask_t):
                    """softmax epilogue + normalize-fold transpose + ctx for
                    one head-pair j2 of query block qb."""
                    p_pair = soft.tile([128, 512], BF16, tag="p_pair")
                    rs = soft.tile([128, 2], F32, tag="rs")
                    rinv = soft.tile([128, 2], F32, tag="rinv")
                    diag = soft.tile([128, 256], BF16, tag="diag")
                    for hi in range(2):
                        nc.scalar.activation(out=p_pair[:, hi * 256:(hi + 1) * 256],
                                             in_=pss[hi], func=AF.Exp,
                                             bias=0.0, scale=1.0)
                        nc.vector.scalar_tensor_tensor(
                            out=p_pair[:, hi * 256:(hi + 1) * 256],
                            in0=p_pair[:, hi * 256:(hi + 1) * 256],
                            scalar=1.0,
                            in1=mask_t[:, hi * 256:(hi + 1) * 256],
                            op0=ALU.mult, op1=ALU.mult,
                            accum_out=rs[:, hi:hi + 1])
                    # one batched reciprocal for both heads; diag stays on
                    # DVE (Pool elementwise measured ~15x slower).
                    nc.vector.reciprocal(rinv, rs)
                    for hi in range(2):
                        nc.vector.tensor_scalar_mul(diag[:, hi * 128:(hi + 1) * 128],
                                                    ident, rinv[:, hi:hi + 1])
                    ptp = ptile(ptr, [128, 512], "ptr")
                    for q4 in range(4):
                        hi = q4 // 2
                        nc.tensor.matmul(ptp[:, q4 * 128:(q4 + 1) * 128],
                                         p_pair[:, q4 * 128:(q4 + 1) * 128],
                                         diag[:, hi * 128:(hi + 1) * 128],
                                         start=True, stop=True)
                    pT = soft.tile([128, 512], BF16, tag="pT")
                    if j2 % 2 == 0:
                        nc.vector.tensor_copy(out=pT, in_=ptp)
                    else:
                        nc.scalar.copy(out=pT, in_=ptp)
                    pc = ptile(pctx, [128, 128], "pctx")
                    for hi, r in enumerate((0, 64)):
                        h = 2 * j2 + hi
                        for half in range(2):
                            kb = qb + half
                            nc.tensor.matmul(pc[r:r + 64, :],
                                             v_sb[:, kb, h * 64:(h + 1) * 64],
                                             pT[:, (hi * 2 + half) * 128:(hi * 2 + half + 1) * 128],
                                             start=(half == 0), stop=(half == 1),
                                             tile_position=(0, r))
                    if j2 % 2 == 0:
                        nc.vector.tensor_copy(out=ctxT[:, j2, qb * 128:(qb + 1) * 128], in_=pc)
                    else:
                        nc.scalar.copy(out=ctxT[:, j2, qb * 128:(qb + 1) * 128], in_=pc)

                with st(name="qkvw", bufs=2) as qkvw, st(name="qaux", bufs=1) as qaux:
                    x_halo = qaux.tile([128, D], BF16)
                    hT = qaux.tile([128, 8, T], BF16)

                    # ident first (2 cheap gpsimd instrs) so the LN1
                    # transposes are never blocked behind DMA descriptor
                    # generation, then a PE warmup covering the wv/x DMA
                    # latency so HAM is released before the first V matmuls.
                    make_identity(nc, ident)
                    nc.vector.memset(eps_tile, LN_EPS)
                    nc.vector.memset(ones1, 1.0)
                    for i in range(72):
                        nc.tensor.transpose(wua, ident, ident)

                    # ---- DMA order (single gpsimd SWDGE queue; ~220GB/s
                    # effective AND ~1us of Pool descriptor-gen per dma_start,
                    # so the order IS the schedule): bvrow first (it is the
                    # K=1 leading accumulation step of every V-proj PSUM
                    # group, so a late bvrow gates the first V matmul), then
                    # x tiles, wv halves woven so V can start early, then
                    # bq/bk + masks (qb0's softmax is merged into the K loop),
                    # then wq/wk column-halves. b1 trails (needed at W2 gelu).
                    xr = ext["x_ext"].rearrange("(t p) d -> p t d", p=128)
                    wv_sb = w1c.tile([128, 8, 1024], BF16, tag="w1c", name="wv")
                    wvr = ext["wv_ext"].rearrange("(k p) n -> p k n", p=128)
                    nc.gpsimd.dma_start(out=bvrow, in_=ext["bv_ext"].rearrange("(a d) -> a d", a=1))
                    nc.gpsimd.dma_start(out=x_halo, in_=xr[:, 0, :])
                    nc.gpsimd.dma_start(out=x_sb[:, 0, :], in_=xr[:, 1, :])
                    nc.gpsimd.dma_start(out=wv_sb[:, :, 0:512], in_=wvr[:, :, 0:512])
                    nc.gpsimd.dma_start(out=x_sb[:, 1, :], in_=xr[:, 2, :])
                    nc.gpsimd.dma_start(out=x_sb[:, 2, :], in_=xr[:, 3, :])
                    nc.gpsimd.dma_start(out=x_sb[:, 3, :], in_=xr[:, 4, :])
                    nc.gpsimd.dma_start(out=wv_sb[:, :, 512:1024], in_=wvr[:, :, 512:1024])
                    nc.gpsimd.dma_start(out=bq_sb, in_=ext["bq_ext"].rearrange("(j p) -> p j", p=128))
                    nc.gpsimd.dma_start(out=bk_sb, in_=ext["bk_ext"].rearrange("(j p) -> p j", p=128))
                    nc.gpsimd.dma_start(out=mask0, in_=ext["mask0_ext"][:, :])
                    nc.gpsimd.dma_start(out=maskr, in_=ext["maskr_ext"][:, :])
                    wq_sb = qkvw.tile([128, 8, D], BF16, tag="wqkv")
                    wqr = ext["wq_ext"].rearrange("(k p) n -> p k n", p=128)
                    for hh in range(2):
                        nc.gpsimd.dma_start(out=wq_sb[:, :, hh * 512:(hh + 1) * 512],
                                            in_=wqr[:, :, hh * 512:(hh + 1) * 512])
                    wk_sb = qkvw.tile([128, 8, D], BF16, tag="wqkv")
                    wkr = ext["wk_ext"].rearrange("(k p) n -> p k n", p=128)
                    for hh in range(2):
                        nc.gpsimd.dma_start(out=wk_sb[:, :, hh * 512:(hh + 1) * 512],
                                            in_=wkr[:, :, hh * 512:(hh + 1) * 512])
                    nc.gpsimd.dma_start(out=b1_sb, in_=ext["b1_ext"].rearrange("(j p) -> p j", p=128))

                    # ---- per-tile pipeline: LN1(t) -> hT(t) -> V-proj(t).
                    # V matmuls give the PE real work from ~13us instead of
                    # idling until all of LN1 is done, and keep HAM warm
                    # through the LN phase. n is the OUTER loop so the V
                    # matmuls needing the second wv half sit late in the PE
                    # FIFO (no head-of-line DMA stall).
                    for t in range(NT):
                        x_t = x_halo if t == 0 else x_sb[:, t - 1, :]
                        mv1 = ln.tile([128, 2], F32, tag="ln_mv")
                        _ln_stats(nc, ln, x_t, mv1)
                        h_t = scr.tile([128, D], BF16, tag="h_t")
                        _ln_apply(nc, ln, x_t, mv1, h_t, eps_tile)
                        for g in range(2):
                            pt = ptile_bf(ptr, [128, 512], "ptr")
                            for jj in range(4):
                                j = g * 4 + jj
                                nc.tensor.transpose(pt[:, jj * 128:(jj + 1) * 128],
                                                    h_t[:, j * 128:(j + 1) * 128], ident)
                            dst = hT[:, g * 4:(g + 1) * 4, t * 128:(t + 1) * 128]
                            if (t * 2 + g) % 2 == 0:
                                nc.vector.tensor_copy(out=dst, in_=pt.rearrange("p (j c) -> p j c", j=4))
                            else:
                                nc.scalar.copy(out=dst, in_=pt.rearrange("p (j c) -> p j c", j=4))
                        n = 0
                        pv = ptile(pmm, [128, 512], "mm")
                        nc.tensor.matmul(pv, ones1, bvrow[:, n * 512:(n + 1) * 512],
                                         start=True, stop=False)
                        for k in range(8):
                            nc.tensor.matmul(pv, hT[:, k, t * 128:(t + 1) * 128],
                                             wv_sb[:, k, n * 512:(n + 1) * 512],
                                             start=False, stop=(k == 7))
                        nc.vector.tensor_copy(out=v_sb[:, t, n * 512:(n + 1) * 512], in_=pv)
                    n = 1
                    for t in range(NT):
                        pv = ptile(pmm, [128, 512], "mm")
                        nc.tensor.matmul(pv, ones1, bvrow[:, n * 512:(n + 1) * 512],
                                         start=True, stop=False)
                        for k in range(8):
                            nc.tensor.matmul(pv, hT[:, k, t * 128:(t + 1) * 128],
                                             wv_sb[:, k, n * 512:(n + 1) * 512],
                                             start=False, stop=(k == 7))
                        nc.scalar.copy(out=v_sb[:, t, n * 512:(n + 1) * 512], in_=pv)

                    # ---- Q projections first (wq lands before wk), then K
                    # interleaved per-j with query-block-0's score emits and
                    # softmax bodies: the K matmul stream is qb0's PE filler
                    # (the same role the W1 bursts play for qb2/3), so the PE
                    # never idles while qb0's DVE/ACT chains run and HAM
                    # stays warm straight through the QKV->attention seam.
                    for j in range(8):
                        pq = ptile(pmm, [128, SEG], "mm")
                        for k in range(8):
                            nc.tensor.matmul(pq, wq_sb[:, k, j * 128:(j + 1) * 128],
                                             hT[:, k, HALO:T], start=(k == 0), stop=(k == 7))
                        nc.scalar.activation(out=qT[:, j, :], in_=pq, func=AF.Identity,
                                             bias=bq_sb[:, j:j + 1], scale=1.0)
                    sq0 = {}
                    for j in range(8):
                        for c0, cn in ((0, 384), (384, 256)):
                            pk = ptile(pmm, [128, cn], "mm")
                            for k in range(8):
                                nc.tensor.matmul(pk, wk_sb[:, k, j * 128:(j + 1) * 128],
                                                 hT[:, k, c0:c0 + cn], start=(k == 0), stop=(k == 7))
                            nc.scalar.activation(out=kT[:, j, c0:c0 + cn], in_=pk, func=AF.Identity,
                                                 bias=bk_sb[:, j:j + 1], scale=1.0)
                        sq0[j] = emit_scores(0, j)
                        if j >= 2:
                            j2_body(0, j - 2, sq0.pop(j - 2), mask0)
                    j2_body(0, 6, sq0.pop(6), mask0)
                    j2_body(0, 7, sq0.pop(7), mask0)

                # ---- attention; LN2 + W1 bursts batched per block pair ----
                # No max-subtraction: scores for this distribution are
                # bounded by ~8 (f32 exp overflows at 88), so exp is safe
                # straight out of PSUM and the row-max reduction is skipped.
                # W1 bursts write RAW pre-activations to gT via DVE copies;
                # the Gelu (with its ACT-table reload) moves to the W2 phase
                # so the attention EXP stream never loses its table.
                with st(name="amisc", bufs=1) as amisc:
                    borow = amisc.tile([1, D], BF16)
                    wo_sb = amisc.tile([128, 8, D], BF16)
                    nc.gpsimd.dma_start(out=borow, in_=ext["bo_ext"].rearrange("(a d) -> a d", a=1))
                    nc.gpsimd.dma_start(out=wo_sb, in_=ext["wo_ext"].rearrange("(k p) n -> p k n", p=128))
                    for cc in range(3):
                        wchunk_dma(w1r[:, :, cc * 1024:(cc + 1) * 1024])

                    def ln2_block(t):
                        h2_t = scr.tile([128, D], BF16, tag="h_t")
                        _ln_apply(nc, ln, x2_sb[:, t, :], mv2[:, t, :], h2_t, eps_tile)
                        for g in range(2):
                            pt = ptile_bf(ptr, [128, 512], "ptr")
                            for jj in range(4):
                                j = g * 4 + jj
                                nc.tensor.transpose(pt[:, jj * 128:(jj + 1) * 128],
                                                    h2_t[:, j * 128:(j + 1) * 128], ident)
                            dst = h2T[:, g * 4:(g + 1) * 4, t * 128:(t + 1) * 128]
                            nc.vector.tensor_copy(out=dst, in_=pt.rearrange("p (j c) -> p j c", j=4))

                    def w1_cols(ci, jlist, tok0, ntok):
                        # W1 columns jdff = ci*8 + jlist over [tok0, tok0+ntok).
                        # Chunks c0..c2 stay RESIDENT until the trailing
                        # token-pair bursts, so W1 streams exactly once from
                        # HBM (v3 re-streamed 8MB). gT evictions alternate
                        # DVE/ACT to keep either off the critical chain.
                        w1t = wtiles[ci]
                        for jj in jlist:
                            jdff = ci * 8 + jj
                            pg = ptile(pmm, [128, ntok], "mm")
                            for k in range(8):
                                nc.tensor.matmul(pg, w1t[:, k, jj * 128:(jj + 1) * 128],
                                                 h2T[:, k, tok0:tok0 + ntok],
                                                 start=(k == 0), stop=(k == 7))
                            if jdff % 2 == 0:
                                nc.vector.tensor_copy(out=gT[:, jdff, tok0:tok0 + ntok], in_=pg)
                            else:
                                nc.scalar.copy(out=gT[:, jdff, tok0:tok0 + ntok], in_=pg)

                    def outproj_ln2(t):
                        # out-projection + residual + LN2 stats for block t;
                        # bn_stats per 512-half fires right behind each
                        # residual add so the stats chain is off the critical
                        # path by the time the second half lands.
                        stats = ln.tile([128, 2, 6], F32, tag="ln_stats")
                        for n in range(2):
                            po = ptile(pmm, [128, 512], "mm")
                            nc.tensor.matmul(po, ones1, borow[:, n * 512:(n + 1) * 512],
                                             start=True, stop=False)
                            for k in range(8):
                                nc.tensor.matmul(po, ctxT[:, k, t * 128:(t + 1) * 128],
                                                 wo_sb[:, k, n * 512:(n + 1) * 512],
                                                 start=False, stop=(k == 7))
                            sl = slice(n * 512, (n + 1) * 512)
                            nc.vector.tensor_add(x2_sb[:, t, sl], po, x_sb[:, t, sl])
                            nc.vector.bn_stats(out=stats[:, n, :], in_=x2_sb[:, t, sl])
                        nc.vector.bn_aggr(out=mv2[:, t, :], in_=stats[:, :, :])

                    # LN2 / W1 work woven into the qb loops as PE filler so
                    # the vector/scalar softmax chains never leave the PE
                    # sparse (HAM re-throttles after ~3.4us of low activity).
                    # W1 token range 0:256 spreads over qb2/qb3 in 1-3 column
                    # slabs per j2; 256:512 runs at the trail when h2T and
                    # all three resident chunks are complete.
                    hooks = {
                        (1, 0): [lambda: ln2_block(0)],
                        (2, 0): [lambda: ln2_block(1)],
                        (2, 1): [lambda: w1_cols(0, (0, 1), 0, 256)],
                        (2, 2): [lambda: w1_cols(0, (2, 3), 0, 256)],
                        (2, 3): [lambda: w1_cols(0, (4, 5), 0, 256)],
                        (2, 4): [lambda: w1_cols(0, (6, 7), 0, 256)],
                        (2, 5): [lambda: w1_cols(1, (0, 1, 2), 0, 256)],
                        (2, 6): [lambda: w1_cols(1, (3, 4, 5), 0, 256)],
                        (2, 7): [lambda: w1_cols(1, (6, 7), 0, 256)],
                        (3, 0): [lambda: ln2_block(2)],
                        (3, 1): [lambda: w1_cols(2, (0,), 0, 256)],
                        (3, 2): [lambda: w1_cols(2, (1,), 0, 256)],
                        (3, 3): [lambda: w1_cols(2, (2,), 0, 256)],
                        (3, 4): [lambda: w1_cols(2, (3,), 0, 256)],
                        (3, 5): [lambda: w1_cols(2, (4, 5), 0, 256)],
                        (3, 6): [lambda: w1_cols(2, (6,), 0, 256)],
                        (3, 7): [lambda: w1_cols(2, (7,), 0, 256)],
                    }

                    # qb0's scores/softmax ran inside the K loop above;
                    # only its out-projection remains.
                    outproj_ln2(0)
                    for qb in range(1, 4):
                        sq = {0: emit_scores(qb, 0), 1: emit_scores(qb, 1)}
                        for j2 in range(8):
                            for h in hooks.get((qb, j2), ()):
                                h()
                            if j2 + 2 < 8:
                                sq[j2 + 2] = emit_scores(qb, j2 + 2)
                            j2_body(qb, j2, sq.pop(j2), maskr)
                        outproj_ln2(qb)

                    # trailing: second token pair over the resident chunks;
                    # chunk3 streams into c0's buffer as soon as c0's last
                    # burst has read it, then covers all 512 tokens at N=512.
                    ln2_block(3)
                    w1_cols(0, tuple(range(8)), 256, 256)
                    wchunk_dma(w1r[:, :, 3 * 1024:4 * 1024])
                    w1_cols(1, tuple(range(8)), 256, 256)
                    w1_cols(2, tuple(range(8)), 256, 256)
                    w1_cols(3, tuple(range(8)), 0, 512)

            # ---- FFN W2: Gelu applied here (one table period, ACT engine is
            # otherwise idle), 8 held PSUM accumulators, w2 streamed in 2MB
            # k-slabs through the same chunk pool (slots already free) ----
            with st(name="pw8", bufs=8, space="PSUM") as pw8, \
                 st(name="wout", bufs=2) as wout:
                b2row = wout.tile([1, D], BF16, tag="b2row")
                nc.gpsimd.dma_start(out=b2row, in_=ext["b2_ext"].rearrange("(a d) -> a d", a=1))
                pys = []
                for ti in range(4):
                    for n in range(2):
                        py = pw8.tile([128, 512], F32, tag="pw8", name=f"py{ti}{n}")
                        nc.tensor.matmul(py, ones1, b2row[:, n * 512:(n + 1) * 512],
                                         start=True, stop=False)
                        pys.append(py)
                outr = ext["out_ext"].rearrange("(t p) d -> p t d", p=128)
                for c in range(4):
                    w2t = w1c.tile([128, 8, 1024], BF16, tag="w1c", name=f"w2c{c}")
                    nc.gpsimd.dma_start(out=w2t, in_=w2r[:, c * 8:(c + 1) * 8, :])
                    for jj in range(8):
                        j = c * 8 + jj
                        nc.scalar.activation(out=gT[:, j, :], in_=gT[:, j, :],
                                             func=AF.Gelu_apprx_tanh,
                                             bias=b1_sb[:, j:j + 1], scale=1.0)
                    if c < 3:
                        for ti in range(4):
                            for n in range(2):
                                py = pys[ti * 2 + n]
                                for kk in range(8):
                                    nc.tensor.matmul(py, gT[:, c * 8 + kk, ti * 128:(ti + 1) * 128],
                                                     w2t[:, kk, n * 512:(n + 1) * 512],
                                                     start=False, stop=False)
                    else:
                        # last slab tile-major with fused drain so the
                        # residual adds and output DMAs overlap the tail
                        for ti in range(4):
                            o_t = wout.tile([128, D], F32, tag="o_t")
                            for n in range(2):
                                py = pys[ti * 2 + n]
                                for kk in range(8):
                                    nc.tensor.matmul(py, gT[:, c * 8 + kk, ti * 128:(ti + 1) * 128],
                                                     w2t[:, kk, n * 512:(n + 1) * 512],
                                                     start=False, stop=(kk == 7))
                                sl = slice(n * 512, (n + 1) * 512)
                                nc.vector.tensor_add(o_t[:, sl], py, x2_sb[:, ti, sl])
                            nc.gpsimd.dma_start(out=outr[:, ti, :], in_=o_t)


def _host_prep(x, Wq, bq, Wk, bk, Wv, bv, Wo, bo, W1, b1, W2, b2,
               ln1_w, ln1_b, ln2_w, ln2_b):
    bf = ml_dtypes.bfloat16
    sc = 1.0 / np.sqrt(DH)
    wq_eff = ((ln1_w[:, None] * Wq) * sc).astype(bf)
    bq_eff = ((bq + ln1_b @ Wq) * sc).astype(np.float32)
    wk_eff = (ln1_w[:, None] * Wk).astype(bf)
    bk_eff = (bk + ln1_b @ Wk).astype(np.float32)
    wv_eff = (ln1_w[:, None] * Wv).astype(bf)
    bv_eff = (bv + ln1_b @ Wv).astype(bf)
    w1_eff = (ln2_w[:, None] * W1).astype(bf)
    b1_eff = (b1 + ln2_b @ W1).astype(np.float32)

    r = np.arange(128)[:, None]
    c = np.arange(128)[None, :]
    left = (c >= r).astype(np.float32)
    diag = (c <= r).astype(np.float32)
    zero = np.zeros((128, 128), np.float32)
    maskr = np.concatenate([left, diag, left, diag], axis=1).astype(bf)
    mask0_halo = np.concatenate([zero, diag, zero, diag], axis=1).astype(bf)

    shared = {
        "wq": wq_eff, "wk": wk_eff, "wv": wv_eff,
        "wo": np.ascontiguousarray(Wo.astype(bf)),
        "w1": w1_eff, "w2": np.ascontiguousarray(W2.astype(bf)),
        "bq": bq_eff, "bk": bk_eff, "bv": bv_eff,
        "bo": bo.astype(bf), "b1": b1_eff, "b2": b2.astype(bf),
        "maskr": maskr,
    }
    in_maps = []
    for core in range(NSEG):
        b_, s_ = core // 4, core % 4
        if s_ == 0:
            seg = np.concatenate(
                [np.zeros((HALO, D), np.float32), x[b_, 0:SEG]], axis=0)
            mask0 = mask0_halo
        else:
            seg = x[b_, s_ * SEG - HALO: (s_ + 1) * SEG]
            mask0 = maskr
        m = dict(shared)
        m["x"] = np.ascontiguousarray(seg.astype(bf))
        m["mask0"] = mask0
        in_maps.append(m)
    return in_maps


def kernel(**inputs):
    from concourse.bass_utils import run_bass_kernel_spmd

    if "nc" not in _CACHED:
        _CACHED["nc"] = _build()
    nc = _CACHED["nc"]

    in_maps = _host_prep(**{k: np.asarray(v) for k, v in inputs.items()})
    trace = bool(int(os.environ.get("KERNEL_TRACE", "0")))
    res = run_bass_kernel_spmd(nc, in_maps, list(range(NSEG)), trace=trace)
    kernel.last_results = res

    x = np.asarray(inputs["x"])
    out = np.empty((B, L, D), np.float32)
    for core in range(NSEG):
        b_, s_ = core // 4, core % 4
        out[b_, s_ * SEG:(s_ + 1) * SEG] = res.results[core]["out"]
    return out



# revision 30
# speedup vs baseline: 1.0449x; 1.0449x over previous
"""Causal local-window (W=128) attention block + FFN, distributed over 8 TRN2
NeuronCores with ZERO collectives.

Sharding: (B=2, L=2048) tokens are split into 8 contiguous segments of 512
tokens (4 per batch element). Each core receives its 512 owned tokens plus a
128-token left halo (zero-padded for the first segment of each batch) and
recomputes the halo's K/V locally — the sliding window (j in [i-128, i]) never
crosses more than 128 tokens back, so no cross-core communication is needed.

Per-core compute layout (v3 — HAM/overlap-optimized):
  - residual stream + LayerNorm stats in token-major [128 tok, 1024] f32
  - matmul activations in feature-major bf16 (PE transposes after each LN)
  - QKV/out-proj/FFN matmuls: bf16 stationary weights, f32 PSUM accumulation;
    bv/bo/b2 biases are folded into the matmuls as a K=1 leading accumulation
    step (ones-row x bias-row), freeing the DVE of broadcast adds.
  - attention: per (head-pair, query-block) two [128,256] f32 PSUM score
    tiles (two single-matmul groups into column ranges of ONE bank hang the
    PE on HW), exp straight out of PSUM (no additive mask), multiplicative
    0/1 mask fused with the row-sum on DVE (scalar_tensor_tensor accum), and
    the softmax 1/rowsum normalization folded into the PE transpose by
    multiplying against diag(rinv) instead of the identity.
  - LN2 + FFN W1 run inside the attention loop: a 16-wide jdff slab of W1
    fires after qb 1, 2 and 3 (x2) as ~14us dense PE bursts — this fills PE
    idle AND re-releases the HAM clock throttle, which otherwise pins the
    whole vector-bound attention phase at K=4/8 (1.2 GHz). W1 streams
    through three 2MB chunk buffers (first pass prefetched, cols 0:2048
    re-streamed for the second token pair: +8MB DMA, well within slack).
  - DMA: every dma_start issues from the gpsimd queue (the SWDGE descriptor
    ring is one shared FIFO per direction; issuing from two engines corrupts
    it and hangs the device). Issue order = arrival order, sized so each
    consumer never waits long.
  - LN scale/bias and the 1/sqrt(dh) score scale are folded into the weight
    matrices on the host, so on-chip LN is pure standardization.
"""

import os
import numpy as np
import ml_dtypes

import concourse.bass as bass
import concourse.mybir as mybir
import concourse.tile as tile
from concourse.masks import make_identity
from bass_rust import ScopedClock

# ---------------------------------------------------------------------------
# Workarounds for the walrus build in this container, which accepts at most
# ONE sync-wait and ONE sync-update per instruction. Tile attaches one wait
# per out-of-date producer clock and one update per consumer engine, so any
# nontrivial Tile kernel violates this. Fix by splitting the extras onto
# standalone InstEventSemaphore instructions on the same engine: waits go
# immediately BEFORE the instruction, updates immediately AFTER (each engine
# executes its stream in order, so semantics are preserved).
_split_counter = [0]


def _split_multi_sync(nc):
    for f in nc.m.functions:
        for bb in f.blocks:
            il = list(bb.instructions)
            new = []
            changed = False
            for inst in il:
                si = inst.sync_info
                waits = list(si.on_wait) if si and si.on_wait else []
                upds = list(si.on_update) if si and si.on_update else []
                if len(waits) > 1:
                    changed = True
                    for w in waits[:-1]:
                        _split_counter[0] += 1
                        new.append(mybir.InstEventSemaphore(
                            name=f"I-wsplit-{_split_counter[0]}",
                            engine=inst.engine, ins=[], outs=[],
                            sync_info=mybir.SyncInfo(on_wait=[w], on_update=[]),
                        ))
                    si.on_wait = [waits[-1]]
                new.append(inst)
                if len(upds) > 1:
                    changed = True
                    si.on_update = [upds[0]]
                    for u in upds[1:]:
                        _split_counter[0] += 1
                        new.append(mybir.InstEventSemaphore(
                            name=f"I-usplit-{_split_counter[0]}",
                            engine=inst.engine, ins=[], outs=[],
                            sync_info=mybir.SyncInfo(on_wait=[], on_update=[u]),
                        ))
            if changed:
                bb.instructions = new


def _patched_drain_and_barrier(self, tick_clock, wait_clock):
    # Tile's kernel-tail drain carries one wait per logical processor; split
    # them into standalone single-wait SP instructions instead.
    nc = self.nc
    drain_inst = nc.sync.drain()
    wait_clock.add_sem_waits(drain_inst.ins, ScopedClock({None: tick_clock.global_clock}))
    si = drain_inst.ins.sync_info
    waits = list(si.on_wait or [])
    if len(waits) > 1:
        si.on_wait = []
        handles = {}
        for s in self.sems.allocated().values():
            nm = getattr(s, 'ant_name', None) or getattr(s, 'name', None)
            handles[nm] = s
        for w in waits:
            assert w.wait_mode == 'sem-ge-imm', w
            nc.sync.wait_ge(handles[w.ant_name], w.wait_value)
    nc.all_engine_barrier()
    assert self.sems is not None
    popped = nc._tile_sem_poison_stack.pop()
    assert popped is self._sem_poison
    nc.clear_and_free_semaphores(list(self.sems.allocated().values()))
    nc.all_engine_barrier()


tile.TileContext._drain_and_barrier = _patched_drain_and_barrier

F32 = mybir.dt.float32
BF16 = mybir.dt.bfloat16
AF = mybir.ActivationFunctionType
ALU = mybir.AluOpType
AX = mybir.AxisListType

B, L, D = 2, 2048, 1024
NH, DH = 16, 64
DFF = 4096
WIN = 128
SEG = 512          # owned tokens per core
HALO = 128
T = SEG + HALO     # 640 local tokens
NT = T // 128      # 5 local token tiles
NSEG = 8           # cores
LN_EPS = 1e-5

_CACHED = {}


def _build():
    nc = bass.Bass()
    # x ships as bf16: halves the critical-path head DMA (~6us); the
    # LN1 stats/apply and the residual adds tolerate 0.4% input rounding
    # easily within the 2e-2 gate (measured ~2.5e-3 end to end).
    x_ext = nc.declare_dram_parameter("x", [T, D], BF16, isOutput=False)
    wq_ext = nc.declare_dram_parameter("wq", [D, D], BF16, isOutput=False)
    wk_ext = nc.declare_dram_parameter("wk", [D, D], BF16, isOutput=False)
    wv_ext = nc.declare_dram_parameter("wv", [D, D], BF16, isOutput=False)
    wo_ext = nc.declare_dram_parameter("wo", [D, D], BF16, isOutput=False)
    w1_ext = nc.declare_dram_parameter("w1", [D, DFF], BF16, isOutput=False)
    w2_ext = nc.declare_dram_parameter("w2", [DFF, D], BF16, isOutput=False)
    bq_ext = nc.declare_dram_parameter("bq", [D], F32, isOutput=False)
    bk_ext = nc.declare_dram_parameter("bk", [D], F32, isOutput=False)
    bv_ext = nc.declare_dram_parameter("bv", [D], BF16, isOutput=False)
    bo_ext = nc.declare_dram_parameter("bo", [D], BF16, isOutput=False)
    b1_ext = nc.declare_dram_parameter("b1", [DFF], F32, isOutput=False)
    b2_ext = nc.declare_dram_parameter("b2", [D], BF16, isOutput=False)
    mask0_ext = nc.declare_dram_parameter("mask0", [128, 512], BF16, isOutput=False)
    maskr_ext = nc.declare_dram_parameter("maskr", [128, 512], BF16, isOutput=False)
    out_ext = nc.declare_dram_parameter("out", [SEG, D], F32, isOutput=True)

    with tile.TileContext(nc) as tc:
        _body(nc, tc, locals())
    if not int(os.environ.get("KERNEL_NO_SPLIT", "0")):
        # walrus-only workaround; CoreSim's race detector rejects the
        # split program, so simcheck.py builds with KERNEL_NO_SPLIT=1.
        _split_multi_sync(nc)
    return nc


def _ln_stats(nc, ln, x_ap, mv_ap):
    """bn stats for one [128, D] f32 tile -> mv_ap [128, 2] (mean, var)."""
    stats = ln.tile([128, 2, 6], F32, tag="ln_stats")
    xr = x_ap.rearrange("p (s f) -> p s f", f=512)
    for s in range(2):
        nc.vector.bn_stats(out=stats[:, s, :], in_=xr[:, s, :])
    nc.vector.bn_aggr(out=mv_ap, in_=stats[:, :, :])


def _ln_apply(nc, ln, x_ap, mv_ap, h_out_ap, eps_tile):
    """h_out = (x - mean) * rsqrt(var + eps), [128, D] f32 -> bf16.

    rstd = exp(-0.5 * ln(var + eps)): Ln and Exp live in the SAME activation
    table set (natural_log_exp_and_others) as the attention-phase Exp, so the
    ACT engine never reloads its function table mid-phase (Sqrt lives in a
    different set and forced a 1.3us ACT_TABLE_LOAD round-trip per LN2 block).
    """
    lnv = ln.tile([128, 1], F32, tag="ln_lnv")
    nc.scalar.activation(out=lnv, in_=mv_ap[:, 1:2], func=AF.Ln, bias=eps_tile, scale=1.0)
    rstd = ln.tile([128, 1], F32, tag="ln_rstd")
    nc.scalar.activation(out=rstd, in_=lnv, func=AF.Exp, bias=0.0, scale=-0.5)
    nmr = ln.tile([128, 1], F32, tag="ln_nmr")
    nc.vector.scalar_tensor_tensor(out=nmr, in0=mv_ap[:, 0:1], scalar=-1.0,
                                   in1=rstd, op0=ALU.mult, op1=ALU.mult)
    nc.scalar.activation(out=h_out_ap, in_=x_ap, func=AF.Identity, bias=nmr, scale=rstd)


def _body(nc, tc, ext):
    st = tc.tile_pool  # shorthand

    with (
        st(name="const", bufs=1) as const,
        st(name="resid", bufs=1) as resid,
        st(name="ln", bufs=3) as ln,
        st(name="scr", bufs=2) as scr,
    ):
        def ptile(pool, shape, tg):
            return pool.tile(shape, F32, tag=tg, name="pst_" + tg)

        def ptile_bf(pool, shape, tg):
            return pool.tile(shape, BF16, tag=tg, name="pstb_" + tg)

        # ---- long-lived tiles ----
        # bf16 residual stream: frees 8KB/partition of SBUF; adds ~0.4%
        # rounding on x2, far inside the 2e-2 gate.
        x2_sb = resid.tile([128, 4, D], BF16)
        mv2 = resid.tile([128, 4, 2], F32)

        ident = const.tile([128, 128], BF16)
        eps_tile = const.tile([128, 1], F32)
        bq_sb = const.tile([128, 8], F32)
        bk_sb = const.tile([128, 8], F32)
        b1_sb = const.tile([128, 32], F32)
        ones1 = const.tile([1, 128], BF16)
        bvrow = const.tile([1, D], BF16)

        with st(name="attnw", bufs=1) as attnw, st(name="fgt", bufs=1) as fgt, \
             st(name="w1c", bufs=3) as w1c:
            x_sb = attnw.tile([128, 4, D], BF16)     # owned tokens only
            qT = attnw.tile([128, 8, SEG], BF16)
            kT = attnw.tile([128, 8, T], BF16)
            v_sb = attnw.tile([128, NT, D], BF16)
            h2T = fgt.tile([128, 8, SEG], BF16)
            gT = fgt.tile([128, 32, SEG], BF16)
            w1r = ext["w1_ext"].rearrange("(k p) n -> p k n", p=128)
            w2r = ext["w2_ext"].rearrange("(c p) n -> p c n", p=128)
            wcnt = [0]
            wtiles = []

            def wchunk_dma(src_ap):
                t_ = w1c.tile([128, 8, 1024], BF16, tag="w1c", name=f"wch{wcnt[0]}")
                wcnt[0] += 1
                nc.gpsimd.dma_start(out=t_, in_=src_ap)
                wtiles.append(t_)


            with (
                st(name="pmm", bufs=2, space="PSUM") as pmm,
                st(name="pscore", bufs=4, space="PSUM") as pscore,
                st(name="ptr", bufs=1, space="PSUM") as ptr,
                st(name="pctx", bufs=1, space="PSUM") as pctx,
                st(name="soft", bufs=4) as soft,
                st(name="atile", bufs=1) as atile,
            ):
                wua = pctx.tile([128, 128], BF16, tag="pctx", name="wua")
                mask0 = atile.tile([128, 512], BF16)
                maskr = atile.tile([128, 512], BF16)
                ctxT = atile.tile([128, 8, SEG], BF16)

                def emit_scores(qb, j2):
                    pss = []
                    for hi, r in enumerate((0, 64)):
                        # separate PSUM tiles: two single-matmul groups
                        # into column ranges of ONE bank hang the PE on
                        # HW (works in sim).
                        ps = ptile(pscore, [128, 256], "psc")
                        nc.tensor.matmul(ps,
                                         qT[r:r + 64, j2, qb * 128:(qb + 1) * 128],
                                         kT[r:r + 64, j2, qb * 128:qb * 128 + 256],
                                         start=True, stop=True)
                        pss.append(ps)
                    return pss

                def j2_body(qb, j2, pss, mask_t):
                    """softmax epilogue + normalize-fold transpose + ctx for
                    one head-pair j2 of query block qb."""
                    p_pair = soft.tile([128, 512], BF16, tag="p_pair")
                    rs = soft.tile([128, 2], F32, tag="rs")
                    rinv = soft.tile([128, 2], F32, tag="rinv")
                    diag = soft.tile([128, 256], BF16, tag="diag")
                    for hi in range(2):
                        nc.scalar.activation(out=p_pair[:, hi * 256:(hi + 1) * 256],
                                             in_=pss[hi], func=AF.Exp,
                                             bias=0.0, scale=1.0)
                        nc.vector.scalar_tensor_tensor(
                            out=p_pair[:, hi * 256:(hi + 1) * 256],
                            in0=p_pair[:, hi * 256:(hi + 1) * 256],
                            scalar=1.0,
                            in1=mask_t[:, hi * 256:(hi + 1) * 256],
                            op0=ALU.mult, op1=ALU.mult,
                            accum_out=rs[:, hi:hi + 1])
                    # one batched reciprocal for both heads; diag stays on
                    # DVE (Pool elementwise measured ~15x slower).
                    nc.vector.reciprocal(rinv, rs)
                    for hi in range(2):
                        nc.vector.tensor_scalar_mul(diag[:, hi * 128:(hi + 1) * 128],
                                                    ident, rinv[:, hi:hi + 1])
                    ptp = ptile(ptr, [128, 512], "ptr")
                    for q4 in range(4):
                        hi = q4 // 2
                        nc.tensor.matmul(ptp[:, q4 * 128:(q4 + 1) * 128],
                                         p_pair[:, q4 * 128:(q4 + 1) * 128],
                                         diag[:, hi * 128:(hi + 1) * 128],
                                         start=True, stop=True)
                    pT = soft.tile([128, 512], BF16, tag="pT")
                    if j2 % 2 == 0:
                        nc.vector.tensor_copy(out=pT, in_=ptp)
                    else:
                        nc.scalar.copy(out=pT, in_=ptp)
                    pc = ptile(pctx, [128, 128], "pctx")
                    for hi, r in enumerate((0, 64)):
                        h = 2 * j2 + hi
                        for half in range(2):
                            kb = qb + half
                            nc.tensor.matmul(pc[r:r + 64, :],
                                             v_sb[:, kb, h * 64:(h + 1) * 64],
                                             pT[:, (hi * 2 + half) * 128:(hi * 2 + half + 1) * 128],
                                             start=(half == 0), stop=(half == 1),
                                             tile_position=(0, r))
                    if j2 % 2 == 0:
                        nc.vector.tensor_copy(out=ctxT[:, j2, qb * 128:(qb + 1) * 128], in_=pc)
                    else:
                        nc.scalar.copy(out=ctxT[:, j2, qb * 128:(qb + 1) * 128], in_=pc)

                with st(name="qkvw", bufs=2) as qkvw, st(name="qaux", bufs=1) as qaux:
                    x_halo = qaux.tile([128, D], BF16)
                    hT = qaux.tile([128, 8, T], BF16)

                    # ident first (2 cheap gpsimd instrs) so the LN1
                    # transposes are never blocked behind DMA descriptor
                    # generation, then a PE warmup covering the wv/x DMA
                    # latency so HAM is released before the first V matmuls.
                    make_identity(nc, ident)
                    nc.vector.memset(eps_tile, LN_EPS)
                    nc.vector.memset(ones1, 1.0)
                    for i in range(72):
                        nc.tensor.transpose(wua, ident, ident)

                    # ---- DMA order (single gpsimd SWDGE queue; ~220GB/s
                    # effective AND ~1us of Pool descriptor-gen per dma_start,
                    # so the order IS the schedule): bvrow first (it is the
                    # K=1 leading accumulation step of every V-proj PSUM
                    # group, so a late bvrow gates the first V matmul), then
                    # x tiles, wv halves woven so V can start early, then
                    # bq/bk + masks (qb0's softmax is merged into the K loop),
                    # then wq/wk column-halves. b1 trails (needed at W2 gelu).
                    xr = ext["x_ext"].rearrange("(t p) d -> p t d", p=128)
                    wv_sb = w1c.tile([128, 8, 1024], BF16, tag="w1c", name="wv")
                    wvr = ext["wv_ext"].rearrange("(k p) n -> p k n", p=128)
                    nc.gpsimd.dma_start(out=bvrow, in_=ext["bv_ext"].rearrange("(a d) -> a d", a=1))
                    nc.gpsimd.dma_start(out=x_halo, in_=xr[:, 0, :])
                    nc.gpsimd.dma_start(out=x_sb[:, 0, :], in_=xr[:, 1, :])
                    nc.gpsimd.dma_start(out=wv_sb[:, :, 0:512], in_=wvr[:, :, 0:512])
                    nc.gpsimd.dma_start(out=x_sb[:, 1, :], in_=xr[:, 2, :])
                    nc.gpsimd.dma_start(out=x_sb[:, 2, :], in_=xr[:, 3, :])
                    nc.gpsimd.dma_start(out=x_sb[:, 3, :], in_=xr[:, 4, :])
                    nc.gpsimd.dma_start(out=wv_sb[:, :, 512:1024], in_=wvr[:, :, 512:1024])
                    nc.gpsimd.dma_start(out=bq_sb, in_=ext["bq_ext"].rearrange("(j p) -> p j", p=128))
                    nc.gpsimd.dma_start(out=bk_sb, in_=ext["bk_ext"].rearrange("(j p) -> p j", p=128))
                    nc.gpsimd.dma_start(out=mask0, in_=ext["mask0_ext"][:, :])
                    nc.gpsimd.dma_start(out=maskr, in_=ext["maskr_ext"][:, :])
                    wq_sb = qkvw.tile([128, 8, D], BF16, tag="wqkv")
                    wqr = ext["wq_ext"].rearrange("(k p) n -> p k n", p=128)
                    for hh in range(2):
                        nc.gpsimd.dma_start(out=wq_sb[:, :, hh * 512:(hh + 1) * 512],
                                            in_=wqr[:, :, hh * 512:(hh + 1) * 512])
                    wk_sb = qkvw.tile([128, 8, D], BF16, tag="wqkv")
                    wkr = ext["wk_ext"].rearrange("(k p) n -> p k n", p=128)
                    for hh in range(2):
                        nc.gpsimd.dma_start(out=wk_sb[:, :, hh * 512:(hh + 1) * 512],
                                            in_=wkr[:, :, hh * 512:(hh + 1) * 512])
                    nc.gpsimd.dma_start(out=b1_sb, in_=ext["b1_ext"].rearrange("(j p) -> p j", p=128))

                    # ---- per-tile pipeline: LN1(t) -> hT(t) -> V-proj(t).
                    # V matmuls give the PE real work from ~13us instead of
                    # idling until all of LN1 is done, and keep HAM warm
                    # through the LN phase. n is the OUTER loop so the V
                    # matmuls needing the second wv half sit late in the PE
                    # FIFO (no head-of-line DMA stall).
                    for t in range(NT):
                        x_t = x_halo if t == 0 else x_sb[:, t - 1, :]
                        mv1 = ln.tile([128, 2], F32, tag="ln_mv")
                        _ln_stats(nc, ln, x_t, mv1)
                        h_t = scr.tile([128, D], BF16, tag="h_t")
                        _ln_apply(nc, ln, x_t, mv1, h_t, eps_tile)
                        for g in range(2):
                            pt = ptile_bf(ptr, [128, 512], "ptr")
                            for jj in range(4):
                                j = g * 4 + jj
                                nc.tensor.transpose(pt[:, jj * 128:(jj + 1) * 128],
                                                    h_t[:, j * 128:(j + 1) * 128], ident)
                            dst = hT[:, g * 4:(g + 1) * 4, t * 128:(t + 1) * 128]
                            if (t * 2 + g) % 2 == 0:
                                nc.vector.tensor_copy(out=dst, in_=pt.rearrange("p (j c) -> p j c", j=4))
                            else:
                                nc.scalar.copy(out=dst, in_=pt.rearrange("p (j c) -> p j c", j=4))
                        n = 0
                        pv = ptile(pmm, [128, 512], "mm")
                        nc.tensor.matmul(pv, ones1, bvrow[:, n * 512:(n + 1) * 512],
                                         start=True, stop=False)
                        for k in range(8):
                            nc.tensor.matmul(pv, hT[:, k, t * 128:(t + 1) * 128],
                                             wv_sb[:, k, n * 512:(n + 1) * 512],
                                             start=False, stop=(k == 7))
                        nc.vector.tensor_copy(out=v_sb[:, t, n * 512:(n + 1) * 512], in_=pv)
                    n = 1
                    for t in range(NT):
                        pv = ptile(pmm, [128, 512], "mm")
                        nc.tensor.matmul(pv, ones1, bvrow[:, n * 512:(n + 1) * 512],
                                         start=True, stop=False)
                        for k in range(8):
                            nc.tensor.matmul(pv, hT[:, k, t * 128:(t + 1) * 128],
                                             wv_sb[:, k, n * 512:(n + 1) * 512],
                                             start=False, stop=(k == 7))
                        nc.scalar.copy(out=v_sb[:, t, n * 512:(n + 1) * 512], in_=pv)

                    # ---- Q projections first (wq lands before wk), then K
                    # interleaved per-j with query-block-0's score emits and
                    # softmax bodies: the K matmul stream is qb0's PE filler
                    # (the same role the W1 bursts play for qb2/3), so the PE
                    # never idles while qb0's DVE/ACT chains run and HAM
                    # stays warm straight through the QKV->attention seam.
                    for j in range(8):
                        pq = ptile(pmm, [128, SEG], "mm")
                        for k in range(8):
                            nc.tensor.matmul(pq, wq_sb[:, k, j * 128:(j + 1) * 128],
                                             hT[:, k, HALO:T], start=(k == 0), stop=(k == 7))
                        nc.scalar.activation(out=qT[:, j, :], in_=pq, func=AF.Identity,
                                             bias=bq_sb[:, j:j + 1], scale=1.0)
                    sq0 = {}
                    for j in range(8):
                        for c0, cn in ((0, 384), (384, 256)):
                            pk = ptile(pmm, [128, cn], "mm")
                            for k in range(8):
                                nc.tensor.matmul(pk, wk_sb[:, k, j * 128:(j + 1) * 128],
                                                 hT[:, k, c0:c0 + cn], start=(k == 0), stop=(k == 7))
                            nc.scalar.activation(out=kT[:, j, c0:c0 + cn], in_=pk, func=AF.Identity,
                                                 bias=bk_sb[:, j:j + 1], scale=1.0)
                        sq0[j] = emit_scores(0, j)
                        if j >= 2:
                            j2_body(0, j - 2, sq0.pop(j - 2), mask0)
                    j2_body(0, 6, sq0.pop(6), mask0)
                    # prefetch qb1's first score pair so the qb1 loop opens
                    # with its epilogue chains already runnable
                    sq_carry = {0: emit_scores(1, 0)}
                    j2_body(0, 7, sq0.pop(7), mask0)
                    sq_carry[1] = emit_scores(1, 1)

                # ---- attention; LN2 + W1 bursts batched per block pair ----
                # No max-subtraction: scores for this distribution are
                # bounded by ~8 (f32 exp overflows at 88), so exp is safe
                # straight out of PSUM and the row-max reduction is skipped.
                # W1 bursts write RAW pre-activations to gT via DVE copies;
                # the Gelu (with its ACT-table reload) moves to the W2 phase
                # so the attention EXP stream never loses its table.
                with st(name="amisc", bufs=1) as amisc:
                    borow = amisc.tile([1, D], BF16)
                    wo_sb = amisc.tile([128, 8, D], BF16)
                    nc.gpsimd.dma_start(out=borow, in_=ext["bo_ext"].rearrange("(a d) -> a d", a=1))
                    nc.gpsimd.dma_start(out=wo_sb, in_=ext["wo_ext"].rearrange("(k p) n -> p k n", p=128))
                    for cc in range(3):
                        wchunk_dma(w1r[:, :, cc * 1024:(cc + 1) * 1024])

                    def ln2_block(t):
                        h2_t = scr.tile([128, D], BF16, tag="h_t")
                        _ln_apply(nc, ln, x2_sb[:, t, :], mv2[:, t, :], h2_t, eps_tile)
                        for g in range(2):
                            pt = ptile_bf(ptr, [128, 512], "ptr")
                            for jj in range(4):
                                j = g * 4 + jj
                                nc.tensor.transpose(pt[:, jj * 128:(jj + 1) * 128],
                                                    h2_t[:, j * 128:(j + 1) * 128], ident)
                            dst = h2T[:, g * 4:(g + 1) * 4, t * 128:(t + 1) * 128]
                            nc.vector.tensor_copy(out=dst, in_=pt.rearrange("p (j c) -> p j c", j=4))

                    def w1_cols(ci, jlist, tok0, ntok):
                        # W1 columns jdff = ci*8 + jlist over [tok0, tok0+ntok).
                        # Chunks c0..c2 stay RESIDENT until the trailing
                        # token-pair bursts, so W1 streams exactly once from
                        # HBM (v3 re-streamed 8MB). gT evictions alternate
                        # DVE/ACT to keep either off the critical chain.
                        w1t = wtiles[ci]
                        for jj in jlist:
                            jdff = ci * 8 + jj
                            pg = ptile(pmm, [128, ntok], "mm")
                            for k in range(8):
                                nc.tensor.matmul(pg, w1t[:, k, jj * 128:(jj + 1) * 128],
                                                 h2T[:, k, tok0:tok0 + ntok],
                                                 start=(k == 0), stop=(k == 7))
                            if jdff % 2 == 0:
                                nc.vector.tensor_copy(out=gT[:, jdff, tok0:tok0 + ntok], in_=pg)
                            else:
                                nc.scalar.copy(out=gT[:, jdff, tok0:tok0 + ntok], in_=pg)

                    def outproj_ln2(t):
                        # out-projection + residual + LN2 stats for block t;
                        # bn_stats per 512-half fires right behind each
                        # residual add so the stats chain is off the critical
                        # path by the time the second half lands.
                        stats = ln.tile([128, 2, 6], F32, tag="ln_stats")
                        for n in range(2):
                            po = ptile(pmm, [128, 512], "mm")
                            nc.tensor.matmul(po, ones1, borow[:, n * 512:(n + 1) * 512],
                                             start=True, stop=False)
                            for k in range(8):
                                nc.tensor.matmul(po, ctxT[:, k, t * 128:(t + 1) * 128],
                                                 wo_sb[:, k, n * 512:(n + 1) * 512],
                                                 start=False, stop=(k == 7))
                            sl = slice(n * 512, (n + 1) * 512)
                            nc.vector.tensor_add(x2_sb[:, t, sl], po, x_sb[:, t, sl])
                            nc.vector.bn_stats(out=stats[:, n, :], in_=x2_sb[:, t, sl])
                        nc.vector.bn_aggr(out=mv2[:, t, :], in_=stats[:, :, :])

                    # LN2 / W1 / out-proj work woven into the qb loops as PE
                    # filler so the vector/scalar softmax chains never leave
                    # the PE sparse (HAM re-throttles after ~3.4us of low
                    # activity), and so no qb->qb seam serializes: each qb's
                    # out-projection + LN2 runs as a mid-loop hook of the
                    # NEXT qb, whose first score pair was prefetched at the
                    # tail of the previous loop.
                    hooks = {
                        (1, 1): [lambda: outproj_ln2(0)],
                        (1, 3): [lambda: ln2_block(0)],
                        (2, 1): [lambda: outproj_ln2(1)],
                        (2, 3): [lambda: ln2_block(1)],
                        (2, 5): [lambda: w1_cols(0, (0, 1, 2), 0, 256)],
                        (2, 6): [lambda: w1_cols(0, (3, 4, 5), 0, 256)],
                        (2, 7): [lambda: w1_cols(0, (6, 7), 0, 256)],
                        (3, 0): [lambda: w1_cols(1, (0, 1), 0, 256)],
                        (3, 1): [lambda: outproj_ln2(2), lambda: w1_cols(1, (2, 3), 0, 256)],
                        (3, 2): [lambda: w1_cols(1, (4, 5), 0, 256)],
                        (3, 3): [lambda: ln2_block(2), lambda: w1_cols(1, (6, 7), 0, 256)],
                        (3, 4): [lambda: w1_cols(2, (0, 1), 0, 256)],
                        (3, 5): [lambda: w1_cols(2, (2, 3), 0, 256)],
                        (3, 6): [lambda: w1_cols(2, (4, 5), 0, 256)],
                        (3, 7): [lambda: w1_cols(2, (6, 7), 0, 256)],
                    }

                    sq = sq_carry
                    for qb in range(1, 4):
                        for j2 in range(8):
                            for h in hooks.get((qb, j2), ()):
                                h()
                            if j2 + 2 < 8:
                                sq[j2 + 2] = emit_scores(qb, j2 + 2)
                            elif qb < 3:
                                # prefetch the next qb's first pair
                                sq[(j2 + 2) % 8] = emit_scores(qb + 1, (j2 + 2) % 8)
                            j2_body(qb, j2, sq.pop(j2), maskr)

                    # trailing: out-proj/LN2 of qb3, with W1 token-half
                    # bursts (block 2 only, N=128) filling the PE while the
                    # LN2(3) serial chain runs; then the second token pair
                    # over the resident chunks; chunk3 streams into c0's
                    # buffer once c0's last burst has read it, and covers
                    # all 512 tokens in one N=512 pass.
                    outproj_ln2(3)
                    w1_cols(0, tuple(range(8)), 256, 128)
                    ln2_block(3)
                    w1_cols(0, tuple(range(8)), 384, 128)
                    wchunk_dma(w1r[:, :, 3 * 1024:4 * 1024])
                    w1_cols(1, tuple(range(8)), 256, 256)
                    w1_cols(2, tuple(range(8)), 256, 256)
                    w1_cols(3, tuple(range(8)), 0, 512)

            # ---- FFN W2: Gelu applied here (one table period, ACT engine is
            # otherwise idle), 8 held PSUM accumulators, w2 streamed in 2MB
            # k-slabs through the same chunk pool (slots already free) ----
            with st(name="pw8", bufs=8, space="PSUM") as pw8, \
                 st(name="wout", bufs=2) as wout:
                b2row = wout.tile([1, D], BF16, tag="b2row")
                nc.gpsimd.dma_start(out=b2row, in_=ext["b2_ext"].rearrange("(a d) -> a d", a=1))
                pys = []
                for ti in range(4):
                    for n in range(2):
                        py = pw8.tile([128, 512], F32, tag="pw8", name=f"py{ti}{n}")
                        nc.tensor.matmul(py, ones1, b2row[:, n * 512:(n + 1) * 512],
                                         start=True, stop=False)
                        pys.append(py)
                outr = ext["out_ext"].rearrange("(t p) d -> p t d", p=128)
                for c in range(4):
                    w2t = w1c.tile([128, 8, 1024], BF16, tag="w1c", name=f"w2c{c}")
                    nc.gpsimd.dma_start(out=w2t, in_=w2r[:, c * 8:(c + 1) * 8, :])
                    for jj in range(8):
                        j = c * 8 + jj
                        nc.scalar.activation(out=gT[:, j, :], in_=gT[:, j, :],
                                             func=AF.Gelu_apprx_tanh,
                                             bias=b1_sb[:, j:j + 1], scale=1.0)
                    if c < 3:
                        for ti in range(4):
                            for n in range(2):
                                py = pys[ti * 2 + n]
                                for kk in range(8):
                                    nc.tensor.matmul(py, gT[:, c * 8 + kk, ti * 128:(ti + 1) * 128],
                                                     w2t[:, kk, n * 512:(n + 1) * 512],
                                                     start=False, stop=False)
                    else:
                        # last slab tile-major with fused drain so the
                        # residual adds and output DMAs overlap the tail
                        for ti in range(4):
                            o_t = wout.tile([128, D], F32, tag="o_t")
                            for n in range(2):
                                py = pys[ti * 2 + n]
                                for kk in range(8):
                                    nc.tensor.matmul(py, gT[:, c * 8 + kk, ti * 128:(ti + 1) * 128],
                                                     w2t[:, kk, n * 512:(n + 1) * 512],
                                                     start=False, stop=(kk == 7))
                                sl = slice(n * 512, (n + 1) * 512)
                                nc.vector.tensor_add(o_t[:, sl], py, x2_sb[:, ti, sl])
                            nc.gpsimd.dma_start(out=outr[:, ti, :], in_=o_t)


def _host_prep(x, Wq, bq, Wk, bk, Wv, bv, Wo, bo, W1, b1, W2, b2,
               ln1_w, ln1_b, ln2_w, ln2_b):
    bf = ml_dtypes.bfloat16
    sc = 1.0 / np.sqrt(DH)
    wq_eff = ((ln1_w[:, None] * Wq) * sc).astype(bf)
    bq_eff = ((bq + ln1_b @ Wq) * sc).astype(np.float32)
    wk_eff = (ln1_w[:, None] * Wk).astype(bf)
    bk_eff = (bk + ln1_b @ Wk).astype(np.float32)
    wv_eff = (ln1_w[:, None] * Wv).astype(bf)
    bv_eff = (bv + ln1_b @ Wv).astype(bf)
    w1_eff = (ln2_w[:, None] * W1).astype(bf)
    b1_eff = (b1 + ln2_b @ W1).astype(np.float32)

    r = np.arange(128)[:, None]
    c = np.arange(128)[None, :]
    left = (c >= r).astype(np.float32)
    diag = (c <= r).astype(np.float32)
    zero = np.zeros((128, 128), np.float32)
    maskr = np.concatenate([left, diag, left, diag], axis=1).astype(bf)
    mask0_halo = np.concatenate([zero, diag, zero, diag], axis=1).astype(bf)

    shared = {
        "wq": wq_eff, "wk": wk_eff, "wv": wv_eff,
        "wo": np.ascontiguousarray(Wo.astype(bf)),
        "w1": w1_eff, "w2": np.ascontiguousarray(W2.astype(bf)),
        "bq": bq_eff, "bk": bk_eff, "bv": bv_eff,
        "bo": bo.astype(bf), "b1": b1_eff, "b2": b2.astype(bf),
        "maskr": maskr,
    }
    in_maps = []
    for core in range(NSEG):
        b_, s_ = core // 4, core % 4
        if s_ == 0:
            seg = np.concatenate(
                [np.zeros((HALO, D), np.float32), x[b_, 0:SEG]], axis=0)
            mask0 = mask0_halo
        else:
            seg = x[b_, s_ * SEG - HALO: (s_ + 1) * SEG]
            mask0 = maskr
        m = dict(shared)
        m["x"] = np.ascontiguousarray(seg.astype(bf))
        m["mask0"] = mask0
        in_maps.append(m)
    return in_maps


def kernel(**inputs):
    from concourse.bass_utils import run_bass_kernel_spmd

    if "nc" not in _CACHED:
        _CACHED["nc"] = _build()
    nc = _CACHED["nc"]

    in_maps = _host_prep(**{k: np.asarray(v) for k, v in inputs.items()})
    trace = bool(int(os.environ.get("KERNEL_TRACE", "0")))
    res = run_bass_kernel_spmd(nc, in_maps, list(range(NSEG)), trace=trace)
    kernel.last_results = res

    x = np.asarray(inputs["x"])
    out = np.empty((B, L, D), np.float32)
    for core in range(NSEG):
        b_, s_ = core // 4, core % 4
        out[b_, s_ * SEG:(s_ + 1) * SEG] = res.results[core]["out"]
    return out

